# revision 24
# baseline (speedup 1.0000x reference)
"""Self-contained TRN2 Bass kernel for the GAT sublayer problem
(nn_GATSubLayer_26998164423437) — v2, DMA/DVE-batched rewrite.

Strategy: dst-bucketed edge-parallel across 8 NeuronCores, no collectives.
Host sorts edges by destination window and buckets per core / per 128-node
dst window.

Device:
  Phase Z: z = h@W plus attention scores, batched 16 tiles per load/store;
  z rows stored to a DRAM table with 264B rows (128 bf16 z + f32 s_src +
  f32 s_dst packed in bf16 slots); a compact [nrows,1] f32 s_dst table is
  written alongside.
  Phase E: per 128-dst window, per-tile indirect DMAs fetch 128 source
  rows each (one index per partition — the HW DGE limit); a batched
  is_equal builds the edge->dst one-hot; s_dst per edge via a one-hot dot
  against a PE-transposed broadcast of the window's s_dst column; softmax
  weights fold into the matmul rhs (exp col 128 gives the denominator);
  out = O^T @ [w*z | exp] accumulated in PSUM, scaled by 1/denom on ACT.
ztab is ping-ponged across repeat iterations so phase Z of iteration k+1
overlaps phase E of iteration k.
"""

import numpy as np
import jax
from jax.sharding import Mesh, PartitionSpec
from jax.experimental.shard_map import shard_map

import concourse.bass as bass
import concourse.mybir as mybir
import concourse.tile as _tile
from concourse.tile import TileContext
from concourse.bass2jax import (
    _bass_exec_p,
    install_neuronx_cc_hook,
    partition_id_tensor,
    fast_dispatch_compile,
)

N_CORES = 8

"""Patches for this walrus build.

The stock tail drain aggregates every live proc-semaphore wait onto a single
Drain instruction (bypassing bass's per-instruction wait-count validation);
walrus's CoreV3 setupSyncWait then rejects it ("Too many sync wait commands").
Emit one Drain per semaphore wait instead. DMA lane semaphores count 16 per
completed DMA, so their wait value is tick*16.
"""


def _split_drain_and_barrier(self, tick_clock, wait_clock):
    nc = self.nc
    clock = tick_clock.global_clock
    sems = wait_clock.sems
    pending = [(proc, tick) for proc, tick in enumerate(clock) if tick > 0]
    if not pending:
        nc.sync.drain()
    for proc, tick in pending:
        sem = sems[proc]
        val = tick * 16 if "DMA" in sem.name else tick
        nc.sync.drain().wait_op(sem, val, "sem-ge")
    nc.all_engine_barrier()
    assert self.sems is not None
    popped = nc._tile_sem_poison_stack.pop()
    assert popped is self._sem_poison
    nc.clear_and_free_semaphores(list(self.sems.allocated().values()))
    nc.all_engine_barrier()


_tile.TileContext._drain_and_barrier = _split_drain_and_barrier


def split_excess_waits(nc, max_cmds=2):
    """This walrus build allows at most 2 sync commands (waits + updates) per
    instruction. Tile's wait assignment can exceed that; peel extra waits onto
    EventSemaphore carriers (2 waits each) inserted just before the
    instruction on the same engine."""
    import concourse.mybir as mybir

    f = nc.m.functions[0]
    n_split = 0
    for bb in f.blocks:
        il = bb.instructions
        i = 0
        while i < len(il):
            ins = il[i]
            si = ins.sync_info
            if si is None:
                i += 1
                continue
            waits = list(si.on_wait or [])
            ups = list(si.on_update or [])
            budget = max(max_cmds - len(ups), 0)
            if len(waits) <= budget:
                i += 1
                continue
            keep = waits[:budget]
            extra = waits[budget:]
            ins.sync_info = mybir.SyncInfo(on_wait=keep, on_update=ups)
            carriers = []
            for j in range(0, len(extra), max_cmds):
                n_split += 1
                carriers.append(
                    mybir.InstEventSemaphore(
                        name=f"waitsplit_{n_split}",
                        engine=ins.engine,
                        sync_info=mybir.SyncInfo(
                            on_wait=extra[j : j + max_cmds], on_update=[]
                        ),
                    )
                )
            il[i:i] = carriers
            i += len(carriers) + 1
    return n_split


P = 128
D = 128
ZROW = 132  # ztab row (bf16 slots): z 0..127, s_src f32 @128..129, s_dst f32 @130..131
NUMF = 129  # zw row: 128 weighted z + exp col (bf16)
G = 16  # z tiles per phase-Z DMA batch


def host_prep(h, W, attn, rel_emb, src, dst, etype, n_cores):
    """Returns (in_maps, meta). All numpy."""
    import ml_dtypes

    bf16 = ml_dtypes.bfloat16
    h = np.asarray(h, np.float32)
    W = np.asarray(W, np.float32)
    attn = np.asarray(attn, np.float32)
    rel_emb = np.asarray(rel_emb, np.float32)
    src = np.asarray(src, np.int32)
    dst = np.asarray(dst, np.int32)
    etype = np.asarray(etype, np.int32)

    N = h.shape[0]
    npc = N // n_cores
    assert npc * n_cores == N
    nwin = (npc + P - 1) // P
    need = (n_cores - 1) * npc + nwin * P
    nrows = ((max(N, need) + P - 1) // P) * P

    wl = (W @ attn[:D]).astype(np.float32)
    wr = (W @ attn[D:]).astype(np.float32)
    w4 = np.zeros((D, 132), np.float32)
    w4[:, :D] = W
    w4[:, D] = wl
    w4[:, D + 1] = wr
    w4 = w4.astype(bf16)

    hT = np.zeros((D, nrows), np.float32)
    hT[:, :N] = np.ascontiguousarray(h.T)
    hT = hT.astype(bf16)

    # ztab row for node n lives at permuted row (n%128)*nzt + n//128 so the
    # phase-Z writeback is one contiguous run per partition
    nzt = nrows // P

    def perm(n):
        return (n % P) * nzt + n // P

    rel_table = rel_emb[:, 0].astype(np.float32).copy()
    rel_table[0] = 0.0
    relv_all = rel_table[etype]

    # ---- bucket edges by (core, window) ----
    core_of = dst // npc
    win_of = (dst % npc) // P
    key = core_of * nwin + win_of
    order = np.argsort(key, kind="stable")
    src_s, dst_s, relv_s = src[order], dst[order], relv_all[order]
    counts = np.bincount(key[order], minlength=n_cores * nwin).reshape(n_cores, nwin)
    tiles_w = np.maximum((counts.max(axis=0) + P - 1) // P, 1).astype(np.int64)
    TT = int(tiles_w.sum())

    in_maps = []
    bounds = np.concatenate([[0], np.cumsum(counts.reshape(-1))])
    iotaR = np.tile(np.arange(P, dtype=np.float32)[None, :], (P, 1))  # [128,128]
    for c in range(n_cores):
        srcidx = np.zeros((P, TT), np.int32)
        dstcol = np.full((P, TT), -1.0, np.float32)
        relv = np.zeros((P, TT), np.float32)
        # permuted row of node c*npc+p; rows for windows w are that + w
        # (c*npc + w*128 + p keeps p' = (c*npc+p)%128 fixed as w varies)
        wnode = np.zeros((P, 1), np.int32)
        wnode[:, 0] = perm(c * npc + np.arange(P))
        toff = 0
        for w in range(nwin):
            k = c * nwin + w
            s, e = bounds[k], bounds[k + 1]
            cnt = e - s
            Tw = int(tiles_w[w])
            se = np.zeros((Tw * P,), np.int32)
            se[:cnt] = perm(src_s[s:e])
            dc = np.full((Tw * P,), -1.0, np.float32)
            dc[:cnt] = (dst_s[s:e] - c * npc - w * P).astype(np.float32)
            rv = np.zeros((Tw * P,), np.float32)
            rv[:cnt] = relv_s[s:e]
            # edge g*128+p -> partition p of tile col toff+g
            srcidx[:, toff : toff + Tw] = se.reshape(Tw, P).T
            dstcol[:, toff : toff + Tw] = dc.reshape(Tw, P).T
            relv[:, toff : toff + Tw] = rv.reshape(Tw, P).T
            toff += Tw
        in_maps.append(
            {
                "hT": hT,
                "w4": w4,
                "srcidx": srcidx,
                "wnode": wnode,
                "dstcol": dstcol.astype(bf16),
                "relv": relv,
                "iota": iotaR.astype(bf16),
                "identb": np.eye(P, dtype=np.float32).astype(bf16),
            }
        )
    meta = {
        "N": N,
        "npc": npc,
        "nwin": nwin,
        "nrows": nrows,
        "tiles_w": [int(t) for t in tiles_w],
        "TT": TT,
        "n_cores": n_cores,
    }
    return in_maps, meta


def build_kernel(meta, repeat=1, for_hw=True, debug=False, gathers_only=False):
    nrows, nwin, TT = meta["nrows"], meta["nwin"], meta["TT"]
    tiles_w = meta["tiles_w"]
    nzt = nrows // P
    TMAXW = max(tiles_w)
    f32 = mybir.dt.float32
    bf16 = mybir.dt.bfloat16

    nc = bass.Bass()
    hT = nc.declare_dram_parameter("hT", [D, nrows], bf16, isOutput=False)
    w4 = nc.declare_dram_parameter("w4", [D, 132], bf16, isOutput=False)
    srcidx = nc.declare_dram_parameter("srcidx", [P, TT], mybir.dt.int32, isOutput=False)
    wnode = nc.declare_dram_parameter("wnode", [P, 1], mybir.dt.int32, isOutput=False)
    dstcol = nc.declare_dram_parameter("dstcol", [P, TT], bf16, isOutput=False)
    relv = nc.declare_dram_parameter("relv", [P, TT], f32, isOutput=False)
    iota = nc.declare_dram_parameter("iota", [P, P], bf16, isOutput=False)
    identb = nc.declare_dram_parameter("identb", [P, P], bf16, isOutput=False)
    out = nc.declare_dram_parameter("out", [nwin * P, D], f32, isOutput=True)
    if debug:
        dzs = nc.declare_dram_parameter("dzs", [P, TT * ZROW], bf16, isOutput=True)
        dsde = nc.declare_dram_parameter("dsde", [P, TT], bf16, isOutput=True)
        dxw = nc.declare_dram_parameter("dxw", [P, TT], f32, isOutput=True)
        dsdw = nc.declare_dram_parameter("dsdw", [P, nwin], f32, isOutput=True)

    nzb = 2 if repeat > 1 else 1
    ztabs = [nc.dram_tensor(f"ztab{i}", [nrows, ZROW], bf16) for i in range(nzb)]
    # +64 pad rows: the batched s_dst stream reads rows perm(base)+w which can
    # touch up to row nrows; pad rows are zero-filled once below
    sdtabs = [nc.dram_tensor(f"sdtab{i}", [nrows + 64, 1], f32) for i in range(nzb)]

    # per-window metadata: (window, tile offset, tile count)
    wmeta = []
    toff = 0
    for w in range(nwin):
        wmeta.append((w, toff, tiles_w[w]))
        toff += tiles_w[w]

    with TileContext(nc) as tc:
        with (
            tc.tile_pool(name="const", bufs=1) as cpool,
            tc.tile_pool(name="zph", bufs=2) as zpool,
            tc.tile_pool(name="zps", bufs=2, space="PSUM") as zpsum,
            tc.tile_pool(name="gat", bufs=4) as gpool,
            tc.tile_pool(name="ewin", bufs=2) as wpool,
            tc.tile_pool(name="epa", bufs=2, space="PSUM") as epsA,
            tc.tile_pool(name="epb", bufs=2, space="PSUM") as epsB,
        ):
            w4sb = cpool.tile([D, 132], bf16, tag="w4")
            nc.sync.dma_start(out=w4sb[:], in_=w4[:])
            iotasb = cpool.tile([P, P], bf16, tag="iota")
            nc.sync.dma_start(out=iotasb[:], in_=iota[:])
            srcsb = cpool.tile([P, TT], mybir.dt.int32, tag="srcidx")
            nc.sync.dma_start(out=srcsb[:], in_=srcidx[:])
            wnsb = cpool.tile([P, 1], mybir.dt.int32, tag="wnode")
            nc.sync.dma_start(out=wnsb[:], in_=wnode[:])
            # zero-fill sdtab pad rows once so the streamed s_dst tail is finite
            zpad = cpool.tile([P, 1], f32, tag="zpad")
            nc.vector.memset(zpad[:], 0.0)
            for _sdt in sdtabs:
                nc.sync.dma_start(out=_sdt[nrows : nrows + 64, :], in_=zpad[0:64, :])
            dcolsb = cpool.tile([P, TT], bf16, tag="dstcol")
            nc.sync.dma_start(out=dcolsb[:], in_=dstcol[:])
            relvsb = cpool.tile([P, TT], f32, tag="relv")
            nc.sync.dma_start(out=relvsb[:], in_=relv[:])
            idsb = cpool.tile([P, P], bf16, tag="identb")
            nc.sync.dma_start(out=idsb[:], in_=identb[:])

            # zero-fill rotating z buffers once so first-iteration pad slots
            # (written to ztab / gathered back) hold finite values
            for _ in range(2):
                zb0 = zpool.tile([P, G * ZROW], bf16, tag="zsb")
                nc.vector.memset(zb0[:], 0.0)

            for _rep in range(repeat):
                ztab = ztabs[_rep % nzb]
                sdtab = sdtabs[_rep % nzb]
                # ---------------- Phase Z ----------------
                for i0 in range(0, nzt, G):
                    g = min(G, nzt - i0)
                    hTg = zpool.tile([P, G * P], bf16, tag="hTg")
                    nc.sync.dma_start(
                        out=hTg[:, : g * P], in_=hT[:, i0 * P : (i0 + g) * P]
                    )
                    zsb = zpool.tile([P, G * ZROW], bf16, tag="zsb")
                    zsb3a = zsb[:].rearrange("p (t c) -> p t c", c=ZROW)
                    for j0 in range(0, g, 2):
                        nb = min(2, g - j0)
                        zp = zpsum.tile([P, 1024], f32, tag="zp")
                        for j in range(j0, j0 + nb):
                            nc.tensor.matmul(
                                out=zp[:, (j - j0) * 512 : (j - j0) * 512 + 132],
                                lhsT=hTg[:, j * P : (j + 1) * P],
                                rhs=w4sb[:],
                                start=True,
                                stop=True,
                            )
                        zp3 = zp[:].rearrange("p (b x) -> p b x", x=512)[:, :nb, :]
                        zsb3 = zsb3a[:, j0 : j0 + nb, :]
                        nc.scalar.activation(
                            out=zsb3[:, :, 0:D],
                            in_=zp3[:, :, 0:D],
                            func=mybir.ActivationFunctionType.Copy,
                        )
                        nc.scalar.activation(
                            out=zsb3[:, :, D : D + 4].bitcast(f32),
                            in_=zp3[:, :, D : D + 2],
                            func=mybir.ActivationFunctionType.Copy,
                        )
                    # permuted layout: node (tile t, part p) -> ztab row p*nzt+t,
                    # so each partition writes one contiguous g*ZROW run
                    nc.sync.dma_start(
                        out=ztab[:].rearrange("(p t) c -> p t c", t=nzt)[
                            :, i0 : i0 + g, :
                        ],
                        in_=zsb3a[:, :g, :],
                    )
                    nc.sync.dma_start(
                        out=sdtab[: nrows, :].rearrange("(p t) c -> p t c", t=nzt)[
                            :, i0 : i0 + g, :
                        ],
                        in_=zsb3a[:, :g, D + 2 : D + 4].bitcast(f32),
                    )

                # ---------------- Phase E ----------------
                # one streamed gather: sdall[p, w] = sdtab[wnode[p] + w]
                # (rows for successive windows are consecutive in the permuted
                # layout; HW streams nwin contiguous values from row wnode[p])
                sdall = gpool.tile([P, nwin], f32, tag="sdall")
                nc.gpsimd.indirect_dma_start(
                    out=sdall[:],
                    out_offset=None,
                    in_=sdtab[:],
                    in_offset=bass.IndirectOffsetOnAxis(ap=wnsb[:], axis=0),
                )
                for (w, toff, Tw) in wmeta:
                    zs = gpool.tile([P, TMAXW * ZROW], bf16, tag="zs")
                    zs3 = zs[:, : Tw * ZROW].rearrange("p (t c) -> p t c", c=ZROW)
                    # HW DGE honors one index per partition per instruction
                    for t in range(Tw):
                        g = toff + t
                        nc.gpsimd.indirect_dma_start(
                            out=zs[:, t * ZROW : (t + 1) * ZROW],
                            out_offset=None,
                            in_=ztab[:],
                            in_offset=bass.IndirectOffsetOnAxis(
                                ap=srcsb[:, g : g + 1], axis=0
                            ),
                        )
                    if gathers_only:
                        ow0 = wpool.tile([P, D], f32, tag="ow")
                        nc.vector.tensor_scalar(
                            out=ow0[:, 0:1], in0=zs[:, 0:1], scalar1=0.0,
                            scalar2=None, op0=mybir.AluOpType.mult,
                        )
                        nc.sync.dma_start(
                            out=out[w * P : (w + 1) * P, 0:1], in_=ow0[:, 0:1]
                        )
                        continue
                    sdwb = wpool.tile([P, 1], bf16, tag="sdwb")
                    nc.scalar.activation(
                        out=sdwb[:],
                        in_=sdall[:, w : w + 1],
                        func=mybir.ActivationFunctionType.Copy,
                    )
                    ptr = epsB.tile([P, P], bf16, tag="ptr")
                    nc.tensor.transpose(
                        out=ptr[:],
                        in_=sdwb[:].to_broadcast([P, P]),
                        identity=idsb[:],
                    )
                    sdrep = wpool.tile([P, P], bf16, tag="sdrep")
                    nc.scalar.activation(
                        out=sdrep[:],
                        in_=ptr[:],
                        func=mybir.ActivationFunctionType.Copy,
                    )
                    # one-hot edge -> dst col, batched over the window
                    Ob = wpool.tile([P, TMAXW * P], bf16, tag="Ob")
                    Ob3 = Ob[:, : Tw * P].rearrange("p (t c) -> p t c", c=P)
                    nc.vector.tensor_tensor(
                        out=Ob3,
                        in0=dcolsb[:, toff : toff + Tw].to_broadcast([P, Tw, P]),
                        in1=iotasb[:]
                        .rearrange("p (a c) -> p a c", a=1)
                        .to_broadcast([P, Tw, P]),
                        op=mybir.AluOpType.is_equal,
                    )
                    # s_dst per edge: one-hot dot against sdrep (batched, 2x)
                    scr = wpool.tile([P, TMAXW * P], bf16, tag="scr")
                    scr3 = scr[:, : Tw * P].rearrange("p (t c) -> p t c", c=P)
                    nc.vector.tensor_tensor(
                        out=scr3,
                        in0=Ob3,
                        in1=sdrep[:]
                        .rearrange("p (a c) -> p a c", a=1)
                        .to_broadcast([P, Tw, P]),
                        op=mybir.AluOpType.mult,
                    )
                    # exact: each (p, t) row of scr has at most one nonzero
                    sde = wpool.tile([P, TMAXW], bf16, tag="sde")
                    with nc.allow_low_precision("one-hot row sum is exact"):
                        nc.vector.tensor_reduce(
                            out=sde[:, :Tw],
                            in_=scr3,
                            axis=mybir.AxisListType.X,
                            op=mybir.AluOpType.add,
                        )
                    # x = s_src + s_dst ; leaky relu
                    ssrcv = zs3[:, :, D : D + 2].bitcast(f32)[:, :, 0]
                    xw = wpool.tile([P, TMAXW], f32, tag="xw")
                    nc.vector.tensor_tensor(
                        out=xw[:, :Tw], in0=sde[:, :Tw], in1=ssrcv,
                        op=mybir.AluOpType.add,
                    )
                    xs = wpool.tile([P, TMAXW], f32, tag="xs")
                    nc.vector.tensor_scalar(
                        out=xs[:, :Tw], in0=xw[:, :Tw], scalar1=0.01,
                        scalar2=None, op0=mybir.AluOpType.mult,
                    )
                    nc.vector.tensor_tensor(
                        out=xw[:, :Tw], in0=xw[:, :Tw], in1=xs[:, :Tw],
                        op=mybir.AluOpType.max,
                    )
                    if debug:
                        nc.sync.dma_start(
                            out=dzs[:, toff * ZROW : (toff + Tw) * ZROW],
                            in_=zs[:, : Tw * ZROW],
                        )
                        nc.sync.dma_start(
                            out=dsde[:, toff : toff + Tw], in_=sde[:, :Tw]
                        )
                        nc.sync.dma_start(
                            out=dxw[:, toff : toff + Tw], in_=xw[:, :Tw]
                        )
                        nc.sync.dma_start(out=dsdw[:, w : w + 1], in_=sdall[:, w : w + 1])
                    # exp into zw col 128; weights into zw cols 0..127
                    zw = wpool.tile([P, TMAXW * NUMF], bf16, tag="zw")
                    zw3 = zw[:, : Tw * NUMF].rearrange("p (t c) -> p t c", c=NUMF)
                    nc.scalar.activation(
                        out=zw3[:, :, D], in_=xw[:, :Tw],
                        func=mybir.ActivationFunctionType.Exp,
                    )
                    wexp = wpool.tile([P, TMAXW], bf16, tag="wexp")
                    nc.vector.tensor_tensor(
                        out=wexp[:, :Tw], in0=zw3[:, :, D],
                        in1=relvsb[:, toff : toff + Tw],
                        op=mybir.AluOpType.mult,
                    )
                    nc.vector.tensor_tensor(
                        out=zw3[:, :, :D],
                        in0=zs3[:, :, :D],
                        in1=wexp[:, :Tw].to_broadcast([P, Tw, P]),
                        op=mybir.AluOpType.mult,
                    )
                    # scatter: pacc[col, :128] = numerator, pacc[col, 128] = denom
                    pacc = epsA.tile([P, NUMF], f32, tag="pacc")
                    for t in range(Tw):
                        nc.tensor.matmul(
                            out=pacc[:],
                            lhsT=Ob[:, t * P : (t + 1) * P],
                            rhs=zw[:, t * NUMF : (t + 1) * NUMF],
                            start=(t == 0),
                            stop=(t == Tw - 1),
                        )
                    dn = wpool.tile([P, 1], f32, tag="dn")
                    nc.vector.tensor_scalar(
                        out=dn[:], in0=pacc[:, D : D + 1], scalar1=1e-30,
                        scalar2=None, op0=mybir.AluOpType.max,
                    )
                    rec = wpool.tile([P, 1], f32, tag="rec")
                    nc.vector.reciprocal(out=rec[:], in_=dn[:])
                    ow = wpool.tile([P, D], f32, tag="ow")
                    nc.scalar.activation(
                        out=ow[:], in_=pacc[:, :D],
                        func=mybir.ActivationFunctionType.Copy, scale=rec[:],
                    )
                    nc.sync.dma_start(
                        out=out[w * P : (w + 1) * P, :], in_=ow[:]
                    )
    if for_hw:
        split_excess_waits(nc)
    return nc


def ref_numpy(h, W, attn, rel_emb, src, dst, etype):
    rel_table = rel_emb.copy()
    rel_table[0] = 0.0
    z = h @ W
    s_src = z @ attn[: W.shape[1]]
    s_dst = z @ attn[W.shape[1] :]
    N = h.shape[0]
    x = s_src[src] + s_dst[dst]
    e = np.where(x > 0, x, 0.01 * x)
    ex = np.exp(e)
    denom = np.zeros(N)
    np.add.at(denom, dst, ex)
    alpha = ex / denom[dst]
    coef = rel_table[etype, 0] * alpha
    out = np.zeros((N, W.shape[1]), np.float64)
    np.add.at(out, dst, coef[:, None] * z[src])
    return out.astype(np.float32)


def make_runner(nc: bass.Bass, in_maps, n_cores: int, chain: int = 1):
    install_neuronx_cc_hook()
    assert nc.dbg_addr is None or not nc.dbg_callbacks

    partition_name = nc.partition_id_tensor.name if nc.partition_id_tensor else None
    in_names, out_names, out_avals = [], [], []
    for alloc in nc.m.functions[0].allocations:
        if not isinstance(alloc, mybir.MemoryLocationSet):
            continue
        name = alloc.memorylocations[0].name
        if alloc.kind == "ExternalInput":
            if name != partition_name and name != (nc.dbg_addr.name if nc.dbg_addr else None):
                in_names.append(name)
        elif alloc.kind == "ExternalOutput":
            out_names.append(name)
            out_avals.append(
                jax.core.ShapedArray(tuple(alloc.tensor_shape), mybir.dt.np(alloc.dtype))
            )
    n_params = len(in_names)
    all_in_names = list(in_names) + list(out_names)
    if nc.dbg_addr is not None:
        in_maps = [{**m, nc.dbg_addr.name: np.zeros((1, 2), np.uint32)} for m in in_maps]
        all_in_names.insert(n_params, nc.dbg_addr.name)
    if partition_name is not None:
        all_in_names.append(partition_name)

    def _body(*args):
        operands = list(args)
        if partition_name is not None:
            operands.append(partition_id_tensor())
        outs = _bass_exec_p.bind(
            *operands,
            out_avals=tuple(out_avals),
            in_names=tuple(all_in_names),
            out_names=tuple(out_names),
            lowering_input_output_aliases=(),
            sim_require_finite=True,
            sim_require_nnan=True,
            nc=nc,
        )
        return tuple(outs)

    devices = jax.devices()[:n_cores]
    mesh = Mesh(np.asarray(devices), ("core",))
    n_outs = len(out_names)

    def _chained(*args):
        params = args[:n_params]
        outs = args[n_params:]
        for _ in range(chain):
            outs = _body(*params, *outs)
        return outs

    def wrapper(*ins):
        return shard_map(
            _chained,
            mesh=mesh,
            in_specs=(PartitionSpec("core"),) * (n_params + n_outs),
            out_specs=(PartitionSpec("core"),) * n_outs,
            check_rep=False,
        )(*ins)

    sh = jax.sharding.NamedSharding(mesh, PartitionSpec("core"))
    concat_in = [
        jax.device_put(
            np.concatenate([np.asarray(in_maps[c][nm]) for c in range(n_cores)], axis=0),
            sh,
        )
        for nm in in_names
    ] + [
        jax.device_put(
            np.zeros((av.shape[0] * n_cores,) + tuple(av.shape[1:]), av.dtype), sh
        )
        for av in out_avals
    ]

    jitted = fast_dispatch_compile(
        lambda: jax.jit(wrapper).lower(*concat_in).compile()
    )

    def run():
        outs = jitted(*concat_in)
        jax.block_until_ready(outs)
        return outs

    def collect(outs):
        res = []
        for c in range(n_cores):
            d = {}
            for i, nm in enumerate(out_names):
                rows = out_avals[i].shape[0]
                d[nm] = np.asarray(outs[i][c * rows : (c + 1) * rows])
            res.append(d)
        return res

    return run, collect


def kernel(**inputs):
    inputs = {k: np.asarray(v) for k, v in inputs.items()}
    in_maps, meta = host_prep(**inputs, n_cores=N_CORES)
    nc = build_kernel(meta)
    run, collect = make_runner(nc, in_maps, N_CORES)
    res = collect(run())
    out = np.concatenate([res[c]["out"][: meta["npc"]] for c in range(N_CORES)], axis=0)
    return out.astype(np.float32)



# revision 25
# speedup vs baseline: 1.0520x; 1.0520x over previous
"""Self-contained TRN2 Bass kernel for the GAT sublayer problem
(nn_GATSubLayer_26998164423437) — v2, DMA/DVE-batched rewrite.

Strategy: dst-bucketed edge-parallel across 8 NeuronCores, no collectives.
Host sorts edges by destination window and buckets per core / per 128-node
dst window.

Device:
  Phase Z: z = h@W plus attention scores, batched 16 tiles per load/store;
  z rows stored to a DRAM table with 264B rows (128 bf16 z + f32 s_src +
  f32 s_dst packed in bf16 slots); a compact [nrows,1] f32 s_dst table is
  written alongside.
  Phase E: per 128-dst window, per-tile indirect DMAs fetch 128 source
  rows each (one index per partition — the HW DGE limit); a batched
  is_equal builds the edge->dst one-hot; s_dst per edge via a one-hot dot
  against a PE-transposed broadcast of the window's s_dst column; softmax
  weights fold into the matmul rhs (exp col 128 gives the denominator);
  out = O^T @ [w*z | exp] accumulated in PSUM, scaled by 1/denom on ACT.
ztab is ping-ponged across repeat iterations so phase Z of iteration k+1
overlaps phase E of iteration k.
"""

import numpy as np
import jax
from jax.sharding import Mesh, PartitionSpec
from jax.experimental.shard_map import shard_map

import concourse.bass as bass
import concourse.mybir as mybir
import concourse.tile as _tile
from concourse.tile import TileContext
from concourse.bass2jax import (
    _bass_exec_p,
    install_neuronx_cc_hook,
    partition_id_tensor,
    fast_dispatch_compile,
)

N_CORES = 8

"""Patches for this walrus build.

The stock tail drain aggregates every live proc-semaphore wait onto a single
Drain instruction (bypassing bass's per-instruction wait-count validation);
walrus's CoreV3 setupSyncWait then rejects it ("Too many sync wait commands").
Emit one Drain per semaphore wait instead. DMA lane semaphores count 16 per
completed DMA, so their wait value is tick*16.
"""


def _split_drain_and_barrier(self, tick_clock, wait_clock):
    nc = self.nc
    clock = tick_clock.global_clock
    sems = wait_clock.sems
    pending = [(proc, tick) for proc, tick in enumerate(clock) if tick > 0]
    if not pending:
        nc.sync.drain()
    for proc, tick in pending:
        sem = sems[proc]
        val = tick * 16 if "DMA" in sem.name else tick
        nc.sync.drain().wait_op(sem, val, "sem-ge")
    nc.all_engine_barrier()
    assert self.sems is not None
    popped = nc._tile_sem_poison_stack.pop()
    assert popped is self._sem_poison
    nc.clear_and_free_semaphores(list(self.sems.allocated().values()))
    nc.all_engine_barrier()


_tile.TileContext._drain_and_barrier = _split_drain_and_barrier


def split_excess_waits(nc, max_cmds=2):
    """This walrus build allows at most 2 sync commands (waits + updates) per
    instruction. Tile's wait assignment can exceed that; peel extra waits onto
    EventSemaphore carriers (2 waits each) inserted just before the
    instruction on the same engine."""
    import concourse.mybir as mybir

    f = nc.m.functions[0]
    n_split = 0
    for bb in f.blocks:
        il = bb.instructions
        i = 0
        while i < len(il):
            ins = il[i]
            si = ins.sync_info
            if si is None:
                i += 1
                continue
            waits = list(si.on_wait or [])
            ups = list(si.on_update or [])
            budget = max(max_cmds - len(ups), 0)
            if len(waits) <= budget:
                i += 1
                continue
            keep = waits[:budget]
            extra = waits[budget:]
            ins.sync_info = mybir.SyncInfo(on_wait=keep, on_update=ups)
            carriers = []
            for j in range(0, len(extra), max_cmds):
                n_split += 1
                carriers.append(
                    mybir.InstEventSemaphore(
                        name=f"waitsplit_{n_split}",
                        engine=ins.engine,
                        sync_info=mybir.SyncInfo(
                            on_wait=extra[j : j + max_cmds], on_update=[]
                        ),
                    )
                )
            il[i:i] = carriers
            i += len(carriers) + 1
    return n_split


P = 128
D = 128
ZROW = 132  # ztab row (bf16 slots): z 0..127, s_src f32 @128..129, s_dst f32 @130..131
NUMF = 129  # zw row: 128 weighted z + exp col (bf16)
G = 16  # z tiles per phase-Z DMA batch


def host_prep(h, W, attn, rel_emb, src, dst, etype, n_cores):
    """Returns (in_maps, meta). All numpy."""
    import ml_dtypes

    bf16 = ml_dtypes.bfloat16
    h = np.asarray(h, np.float32)
    W = np.asarray(W, np.float32)
    attn = np.asarray(attn, np.float32)
    rel_emb = np.asarray(rel_emb, np.float32)
    src = np.asarray(src, np.int32)
    dst = np.asarray(dst, np.int32)
    etype = np.asarray(etype, np.int32)

    N = h.shape[0]
    npc = N // n_cores
    assert npc * n_cores == N
    nwin = (npc + P - 1) // P
    need = (n_cores - 1) * npc + nwin * P
    nrows = ((max(N, need) + P - 1) // P) * P

    wl = (W @ attn[:D]).astype(np.float32)
    wr = (W @ attn[D:]).astype(np.float32)
    w4 = np.zeros((D, 132), np.float32)
    w4[:, :D] = W
    w4[:, D] = wl
    w4[:, D + 1] = wr
    w4 = w4.astype(bf16)

    hT = np.zeros((D, nrows), np.float32)
    hT[:, :N] = np.ascontiguousarray(h.T)
    hT = hT.astype(bf16)

    # ztab row for node n lives at permuted row (n%128)*nzt + n//128 so the
    # phase-Z writeback is one contiguous run per partition
    nzt = nrows // P

    def perm(n):
        return (n % P) * nzt + n // P

    rel_table = rel_emb[:, 0].astype(np.float32).copy()
    rel_table[0] = 0.0
    relv_all = rel_table[etype]

    # ---- bucket edges by (core, window) ----
    core_of = dst // npc
    win_of = (dst % npc) // P
    key = core_of * nwin + win_of
    order = np.argsort(key, kind="stable")
    src_s, dst_s, relv_s = src[order], dst[order], relv_all[order]
    counts = np.bincount(key[order], minlength=n_cores * nwin).reshape(n_cores, nwin)
    tiles_w = np.maximum((counts.max(axis=0) + P - 1) // P, 1).astype(np.int64)
    TT = int(tiles_w.sum())

    in_maps = []
    bounds = np.concatenate([[0], np.cumsum(counts.reshape(-1))])
    iotaR = np.tile(np.arange(P, dtype=np.float32)[None, :], (P, 1))  # [128,128]
    for c in range(n_cores):
        srcidx = np.zeros((P, TT), np.int32)
        dstcol = np.full((P, TT), -1.0, np.float32)
        relv = np.zeros((P, TT), np.float32)
        # permuted row of node c*npc+p; rows for windows w are that + w
        # (c*npc + w*128 + p keeps p' = (c*npc+p)%128 fixed as w varies)
        wnode = np.zeros((P, 1), np.int32)
        wnode[:, 0] = perm(c * npc + np.arange(P))
        toff = 0
        for w in range(nwin):
            k = c * nwin + w
            s, e = bounds[k], bounds[k + 1]
            cnt = e - s
            Tw = int(tiles_w[w])
            se = np.zeros((Tw * P,), np.int32)
            se[:cnt] = perm(src_s[s:e])
            dc = np.full((Tw * P,), -1.0, np.float32)
            dc[:cnt] = (dst_s[s:e] - c * npc - w * P).astype(np.float32)
            rv = np.zeros((Tw * P,), np.float32)
            rv[:cnt] = relv_s[s:e]
            # edge g*128+p -> partition p of tile col toff+g
            srcidx[:, toff : toff + Tw] = se.reshape(Tw, P).T
            dstcol[:, toff : toff + Tw] = dc.reshape(Tw, P).T
            relv[:, toff : toff + Tw] = rv.reshape(Tw, P).T
            toff += Tw
        in_maps.append(
            {
                "hT": hT,
                "w4": w4,
                "srcidx": srcidx,
                "wnode": wnode,
                "dstcol": dstcol.astype(bf16),
                "relv": relv,
                "iota": iotaR.astype(bf16),
                "identb": np.eye(P, dtype=np.float32).astype(bf16),
            }
        )
    meta = {
        "N": N,
        "npc": npc,
        "nwin": nwin,
        "nrows": nrows,
        "tiles_w": [int(t) for t in tiles_w],
        "TT": TT,
        "n_cores": n_cores,
    }
    return in_maps, meta


def build_kernel(meta, repeat=1, for_hw=True, debug=False, gathers_only=False):
    nrows, nwin, TT = meta["nrows"], meta["nwin"], meta["TT"]
    tiles_w = meta["tiles_w"]
    nzt = nrows // P
    TMAXW = max(tiles_w)
    f32 = mybir.dt.float32
    bf16 = mybir.dt.bfloat16

    nc = bass.Bass()
    hT = nc.declare_dram_parameter("hT", [D, nrows], bf16, isOutput=False)
    w4 = nc.declare_dram_parameter("w4", [D, 132], bf16, isOutput=False)
    srcidx = nc.declare_dram_parameter("srcidx", [P, TT], mybir.dt.int32, isOutput=False)
    wnode = nc.declare_dram_parameter("wnode", [P, 1], mybir.dt.int32, isOutput=False)
    dstcol = nc.declare_dram_parameter("dstcol", [P, TT], bf16, isOutput=False)
    relv = nc.declare_dram_parameter("relv", [P, TT], f32, isOutput=False)
    iota = nc.declare_dram_parameter("iota", [P, P], bf16, isOutput=False)
    identb = nc.declare_dram_parameter("identb", [P, P], bf16, isOutput=False)
    out = nc.declare_dram_parameter("out", [nwin * P, D], f32, isOutput=True)
    if debug:
        dzs = nc.declare_dram_parameter("dzs", [P, TT * ZROW], bf16, isOutput=True)
        dsde = nc.declare_dram_parameter("dsde", [P, TT], bf16, isOutput=True)
        dxw = nc.declare_dram_parameter("dxw", [P, TT], f32, isOutput=True)
        dsdw = nc.declare_dram_parameter("dsdw", [P, nwin], f32, isOutput=True)

    nzb = 2 if repeat > 1 else 1
    ztabs = [nc.dram_tensor(f"ztab{i}", [nrows, ZROW], bf16) for i in range(nzb)]
    # +64 pad rows: the batched s_dst stream reads rows perm(base)+w which can
    # touch up to row nrows; pad rows are zero-filled once below
    sdtabs = [nc.dram_tensor(f"sdtab{i}", [nrows + 64, 1], f32) for i in range(nzb)]

    # per-window metadata: (window, tile offset, tile count)
    wmeta = []
    toff = 0
    for w in range(nwin):
        wmeta.append((w, toff, tiles_w[w]))
        toff += tiles_w[w]

    with TileContext(nc) as tc:
        with (
            tc.tile_pool(name="const", bufs=1) as cpool,
            tc.tile_pool(name="zph", bufs=2) as zpool,
            tc.tile_pool(name="zps", bufs=2, space="PSUM") as zpsum,
            tc.tile_pool(name="gat", bufs=4) as gpool,
            tc.tile_pool(name="ewin", bufs=2) as wpool,
            tc.tile_pool(name="epa", bufs=2, space="PSUM") as epsA,
            tc.tile_pool(name="epb", bufs=2, space="PSUM") as epsB,
        ):
            w4sb = cpool.tile([D, 132], bf16, tag="w4")
            nc.sync.dma_start(out=w4sb[:], in_=w4[:])
            iotasb = cpool.tile([P, P], bf16, tag="iota")
            nc.sync.dma_start(out=iotasb[:], in_=iota[:])
            srcsb = cpool.tile([P, TT], mybir.dt.int32, tag="srcidx")
            nc.sync.dma_start(out=srcsb[:], in_=srcidx[:])
            wnsb = cpool.tile([P, 1], mybir.dt.int32, tag="wnode")
            nc.sync.dma_start(out=wnsb[:], in_=wnode[:])
            # zero-fill sdtab pad rows once so the streamed s_dst tail is finite
            zpad = cpool.tile([P, 1], f32, tag="zpad")
            nc.vector.memset(zpad[:], 0.0)
            for _sdt in sdtabs:
                nc.sync.dma_start(out=_sdt[nrows : nrows + 64, :], in_=zpad[0:64, :])
            dcolsb = cpool.tile([P, TT], bf16, tag="dstcol")
            nc.sync.dma_start(out=dcolsb[:], in_=dstcol[:])
            relvsb = cpool.tile([P, TT], f32, tag="relv")
            nc.sync.dma_start(out=relvsb[:], in_=relv[:])
            idsb = cpool.tile([P, P], bf16, tag="identb")
            nc.sync.dma_start(out=idsb[:], in_=identb[:])

            # zero-fill rotating z buffers once so first-iteration pad slots
            # (written to ztab / gathered back) hold finite values
            for _ in range(2):
                zb0 = zpool.tile([P, G * ZROW], bf16, tag="zsb")
                nc.vector.memset(zb0[:], 0.0)

            for _rep in range(repeat):
                ztab = ztabs[_rep % nzb]
                sdtab = sdtabs[_rep % nzb]
                # ---------------- Phase Z ----------------
                for i0 in range(0, nzt, G):
                    g = min(G, nzt - i0)
                    hTg = zpool.tile([P, G * P], bf16, tag="hTg")
                    nc.sync.dma_start(
                        out=hTg[:, : g * P], in_=hT[:, i0 * P : (i0 + g) * P]
                    )
                    zsb = zpool.tile([P, G * ZROW], bf16, tag="zsb")
                    zsb3a = zsb[:].rearrange("p (t c) -> p t c", c=ZROW)
                    for j0 in range(0, g, 2):
                        nb = min(2, g - j0)
                        zp = zpsum.tile([P, 1024], f32, tag="zp")
                        for j in range(j0, j0 + nb):
                            nc.tensor.matmul(
                                out=zp[:, (j - j0) * 512 : (j - j0) * 512 + 132],
                                lhsT=hTg[:, j * P : (j + 1) * P],
                                rhs=w4sb[:],
                                start=True,
                                stop=True,
                            )
                        zp3 = zp[:].rearrange("p (b x) -> p b x", x=512)[:, :nb, :]
                        zsb3 = zsb3a[:, j0 : j0 + nb, :]
                        nc.scalar.activation(
                            out=zsb3[:, :, 0:D],
                            in_=zp3[:, :, 0:D],
                            func=mybir.ActivationFunctionType.Copy,
                        )
                        nc.scalar.activation(
                            out=zsb3[:, :, D : D + 4].bitcast(f32),
                            in_=zp3[:, :, D : D + 2],
                            func=mybir.ActivationFunctionType.Copy,
                        )
                    # permuted layout: node (tile t, part p) -> ztab row p*nzt+t,
                    # so each partition writes one contiguous g*ZROW run
                    nc.sync.dma_start(
                        out=ztab[:].rearrange("(p t) c -> p t c", t=nzt)[
                            :, i0 : i0 + g, :
                        ],
                        in_=zsb3a[:, :g, :],
                    )
                    nc.sync.dma_start(
                        out=sdtab[: nrows, :].rearrange("(p t) c -> p t c", t=nzt)[
                            :, i0 : i0 + g, :
                        ],
                        in_=zsb3a[:, :g, D + 2 : D + 4].bitcast(f32),
                    )

                # ---------------- Phase E ----------------
                # one streamed gather: sdall[p, w] = sdtab[wnode[p] + w]
                # (rows for successive windows are consecutive in the permuted
                # layout; HW streams nwin contiguous values from row wnode[p])
                sdall = gpool.tile([P, nwin], f32, tag="sdall")
                nc.gpsimd.indirect_dma_start(
                    out=sdall[:],
                    out_offset=None,
                    in_=sdtab[:],
                    in_offset=bass.IndirectOffsetOnAxis(ap=wnsb[:], axis=0),
                )
                for (w, toff, Tw) in wmeta:
                    zs = gpool.tile([P, TMAXW * ZROW], bf16, tag="zs")
                    zs3 = zs[:, : Tw * ZROW].rearrange("p (t c) -> p t c", c=ZROW)
                    # HW DGE honors one index per partition per instruction
                    for t in range(Tw):
                        g = toff + t
                        nc.gpsimd.indirect_dma_start(
                            out=zs[:, t * ZROW : (t + 1) * ZROW],
                            out_offset=None,
                            in_=ztab[:],
                            in_offset=bass.IndirectOffsetOnAxis(
                                ap=srcsb[:, g : g + 1], axis=0
                            ),
                        )
                    if gathers_only:
                        ow0 = wpool.tile([P, D], f32, tag="ow")
                        nc.vector.tensor_scalar(
                            out=ow0[:, 0:1], in0=zs[:, 0:1], scalar1=0.0,
                            scalar2=None, op0=mybir.AluOpType.mult,
                        )
                        nc.sync.dma_start(
                            out=out[w * P : (w + 1) * P, 0:1], in_=ow0[:, 0:1]
                        )
                        continue
                    sdwb = wpool.tile([P, 1], bf16, tag="sdwb")
                    nc.scalar.activation(
                        out=sdwb[:],
                        in_=sdall[:, w : w + 1],
                        func=mybir.ActivationFunctionType.Copy,
                    )
                    ptr = epsB.tile([P, P], bf16, tag="ptr")
                    nc.tensor.transpose(
                        out=ptr[:],
                        in_=sdwb[:].to_broadcast([P, P]),
                        identity=idsb[:],
                    )
                    sdrep = wpool.tile([P, P], bf16, tag="sdrep")
                    nc.scalar.activation(
                        out=sdrep[:],
                        in_=ptr[:],
                        func=mybir.ActivationFunctionType.Copy,
                    )
                    # one-hot edge -> dst col, batched over the window
                    Ob = wpool.tile([P, TMAXW * P], bf16, tag="Ob")
                    Ob3 = Ob[:, : Tw * P].rearrange("p (t c) -> p t c", c=P)
                    nc.vector.tensor_tensor(
                        out=Ob3,
                        in0=dcolsb[:, toff : toff + Tw].to_broadcast([P, Tw, P]),
                        in1=iotasb[:]
                        .rearrange("p (a c) -> p a c", a=1)
                        .to_broadcast([P, Tw, P]),
                        op=mybir.AluOpType.is_equal,
                    )
                    # s_dst per edge: one-hot dot against sdrep (batched, 2x)
                    scr = wpool.tile([P, TMAXW * P], bf16, tag="scr")
                    scr3 = scr[:, : Tw * P].rearrange("p (t c) -> p t c", c=P)
                    nc.vector.tensor_tensor(
                        out=scr3,
                        in0=Ob3,
                        in1=sdrep[:]
                        .rearrange("p (a c) -> p a c", a=1)
                        .to_broadcast([P, Tw, P]),
                        op=mybir.AluOpType.mult,
                    )
                    # exact: each (p, t) row of scr has at most one nonzero
                    sde = wpool.tile([P, TMAXW], bf16, tag="sde")
                    with nc.allow_low_precision("one-hot row sum is exact"):
                        nc.vector.tensor_reduce(
                            out=sde[:, :Tw],
                            in_=scr3,
                            axis=mybir.AxisListType.X,
                            op=mybir.AluOpType.add,
                        )
                    # x = s_src + s_dst ; leaky relu
                    ssrcv = zs3[:, :, D : D + 2].bitcast(f32)[:, :, 0]
                    xw = wpool.tile([P, TMAXW], f32, tag="xw")
                    nc.vector.tensor_tensor(
                        out=xw[:, :Tw], in0=sde[:, :Tw], in1=ssrcv,
                        op=mybir.AluOpType.add,
                    )
                    xs = wpool.tile([P, TMAXW], f32, tag="xs")
                    nc.vector.tensor_scalar(
                        out=xs[:, :Tw], in0=xw[:, :Tw], scalar1=0.01,
                        scalar2=None, op0=mybir.AluOpType.mult,
                    )
                    nc.vector.tensor_tensor(
                        out=xw[:, :Tw], in0=xw[:, :Tw], in1=xs[:, :Tw],
                        op=mybir.AluOpType.max,
                    )
                    if debug:
                        nc.sync.dma_start(
                            out=dzs[:, toff * ZROW : (toff + Tw) * ZROW],
                            in_=zs[:, : Tw * ZROW],
                        )
                        nc.sync.dma_start(
                            out=dsde[:, toff : toff + Tw], in_=sde[:, :Tw]
                        )
                        nc.sync.dma_start(
                            out=dxw[:, toff : toff + Tw], in_=xw[:, :Tw]
                        )
                        nc.sync.dma_start(out=dsdw[:, w : w + 1], in_=sdall[:, w : w + 1])
                    # exp into zw col 128; weights into zw cols 0..127
                    zw = wpool.tile([P, TMAXW * NUMF], bf16, tag="zw")
                    zw3 = zw[:, : Tw * NUMF].rearrange("p (t c) -> p t c", c=NUMF)
                    nc.scalar.activation(
                        out=zw3[:, :, D], in_=xw[:, :Tw],
                        func=mybir.ActivationFunctionType.Exp,
                    )
                    wexp = wpool.tile([P, TMAXW], bf16, tag="wexp")
                    nc.vector.tensor_tensor(
                        out=wexp[:, :Tw], in0=zw3[:, :, D],
                        in1=relvsb[:, toff : toff + Tw],
                        op=mybir.AluOpType.mult,
                    )
                    nc.vector.tensor_tensor(
                        out=zw3[:, :, :D],
                        in0=zs3[:, :, :D],
                        in1=wexp[:, :Tw].to_broadcast([P, Tw, P]),
                        op=mybir.AluOpType.mult,
                    )
                    # scatter: pacc[col, :128] = numerator, pacc[col, 128] = denom
                    pacc = epsA.tile([P, NUMF], f32, tag="pacc")
                    for t in range(Tw):
                        nc.tensor.matmul(
                            out=pacc[:],
                            lhsT=Ob[:, t * P : (t + 1) * P],
                            rhs=zw[:, t * NUMF : (t + 1) * NUMF],
                            start=(t == 0),
                            stop=(t == Tw - 1),
                        )
                    dn = wpool.tile([P, 1], f32, tag="dn")
                    nc.vector.tensor_scalar(
                        out=dn[:], in0=pacc[:, D : D + 1], scalar1=1e-30,
                        scalar2=None, op0=mybir.AluOpType.max,
                    )
                    rec = wpool.tile([P, 1], f32, tag="rec")
                    nc.vector.reciprocal(out=rec[:], in_=dn[:])
                    ow = wpool.tile([P, D], f32, tag="ow")
                    nc.scalar.activation(
                        out=ow[:], in_=pacc[:, :D],
                        func=mybir.ActivationFunctionType.Copy, scale=rec[:],
                    )
                    # issue from the ACT queue: its wait (ow, the preceding ACT
                    # inst) is already satisfied at queue head. On the sync
                    # queue these 49 writes head-of-line block phase Z(k+1)'s
                    # loads behind each window's full compute pipeline.
                    nc.scalar.dma_start(
                        out=out[w * P : (w + 1) * P, :], in_=ow[:]
                    )
    if for_hw:
        split_excess_waits(nc)
    return nc


def ref_numpy(h, W, attn, rel_emb, src, dst, etype):
    rel_table = rel_emb.copy()
    rel_table[0] = 0.0
    z = h @ W
    s_src = z @ attn[: W.shape[1]]
    s_dst = z @ attn[W.shape[1] :]
    N = h.shape[0]
    x = s_src[src] + s_dst[dst]
    e = np.where(x > 0, x, 0.01 * x)
    ex = np.exp(e)
    denom = np.zeros(N)
    np.add.at(denom, dst, ex)
    alpha = ex / denom[dst]
    coef = rel_table[etype, 0] * alpha
    out = np.zeros((N, W.shape[1]), np.float64)
    np.add.at(out, dst, coef[:, None] * z[src])
    return out.astype(np.float32)


def make_runner(nc: bass.Bass, in_maps, n_cores: int, chain: int = 1):
    install_neuronx_cc_hook()
    assert nc.dbg_addr is None or not nc.dbg_callbacks

    partition_name = nc.partition_id_tensor.name if nc.partition_id_tensor else None
    in_names, out_names, out_avals = [], [], []
    for alloc in nc.m.functions[0].allocations:
        if not isinstance(alloc, mybir.MemoryLocationSet):
            continue
        name = alloc.memorylocations[0].name
        if alloc.kind == "ExternalInput":
            if name != partition_name and name != (nc.dbg_addr.name if nc.dbg_addr else None):
                in_names.append(name)
        elif alloc.kind == "ExternalOutput":
            out_names.append(name)
            out_avals.append(
                jax.core.ShapedArray(tuple(alloc.tensor_shape), mybir.dt.np(alloc.dtype))
            )
    n_params = len(in_names)
    all_in_names = list(in_names) + list(out_names)
    if nc.dbg_addr is not None:
        in_maps = [{**m, nc.dbg_addr.name: np.zeros((1, 2), np.uint32)} for m in in_maps]
        all_in_names.insert(n_params, nc.dbg_addr.name)
    if partition_name is not None:
        all_in_names.append(partition_name)

    def _body(*args):
        operands = list(args)
        if partition_name is not None:
            operands.append(partition_id_tensor())
        outs = _bass_exec_p.bind(
            *operands,
            out_avals=tuple(out_avals),
            in_names=tuple(all_in_names),
            out_names=tuple(out_names),
            lowering_input_output_aliases=(),
            sim_require_finite=True,
            sim_require_nnan=True,
            nc=nc,
        )
        return tuple(outs)

    devices = jax.devices()[:n_cores]
    mesh = Mesh(np.asarray(devices), ("core",))
    n_outs = len(out_names)

    def _chained(*args):
        params = args[:n_params]
        outs = args[n_params:]
        for _ in range(chain):
            outs = _body(*params, *outs)
        return outs

    def wrapper(*ins):
        return shard_map(
            _chained,
            mesh=mesh,
            in_specs=(PartitionSpec("core"),) * (n_params + n_outs),
            out_specs=(PartitionSpec("core"),) * n_outs,
            check_rep=False,
        )(*ins)

    sh = jax.sharding.NamedSharding(mesh, PartitionSpec("core"))
    concat_in = [
        jax.device_put(
            np.concatenate([np.asarray(in_maps[c][nm]) for c in range(n_cores)], axis=0),
            sh,
        )
        for nm in in_names
    ] + [
        jax.device_put(
            np.zeros((av.shape[0] * n_cores,) + tuple(av.shape[1:]), av.dtype), sh
        )
        for av in out_avals
    ]

    jitted = fast_dispatch_compile(
        lambda: jax.jit(wrapper).lower(*concat_in).compile()
    )

    def run():
        outs = jitted(*concat_in)
        jax.block_until_ready(outs)
        return outs

    def collect(outs):
        res = []
        for c in range(n_cores):
            d = {}
            for i, nm in enumerate(out_names):
                rows = out_avals[i].shape[0]
                d[nm] = np.asarray(outs[i][c * rows : (c + 1) * rows])
            res.append(d)
        return res

    return run, collect


def kernel(**inputs):
    inputs = {k: np.asarray(v) for k, v in inputs.items()}
    in_maps, meta = host_prep(**inputs, n_cores=N_CORES)
    nc = build_kernel(meta)
    run, collect = make_runner(nc, in_maps, N_CORES)
    res = collect(run())
    out = np.concatenate([res[c]["out"][: meta["npc"]] for c in range(N_CORES)], axis=0)
    return out.astype(np.float32)

